# revision 25
# baseline (speedup 1.0000x reference)
"""Bass/Trainium2 kernel for nn_Attention (additive attention + weighted sum).

Computation (reference):
    enc  = encoder_outputs.transpose(1, 0, 2)              # [B, S, E]
    z    = enc @ w_e.T + hidden @ w_h.T + attn_b           # [B, S, O]
    att  = softmax(tanh(z) @ v, axis=S)                    # [B, S]
    out  = att @ enc                                       # [B, E]

Sharding: data-parallel over batch — 8 cores x 4 batches each.
Host precomputes hidden @ w_h.T + attn_b (0.1% of FLOPs) and ships the
encoder slice in [b, e, s] layout (contraction over e needs e on
partitions for the big matmul).

Per core, per batch b, per 512-wide s-chunk:
  PE:   energy = tanh(w_e @ enc_chunk + bias)  (fp32r matmuls, full rate)
        scores_chunk = v . energy              (M=1 matmuls)
  then a chunk-local softmax (exp at the chunk max) and the chunk's
  attention-weighted sum run on ACT/DVE/GpSimd, reusing the SAME encoder
  tile from SBUF (multiply + free-axis reduce) while the PE streams the
  next chunk. A final recombine rescales the four chunk-local partial
  sums by exp(m_c - M)/den. The encoder is read from HBM exactly once
  and the PE does nothing but the two contractions.
"""

import numpy as np
from contextlib import ExitStack

# Problem shapes (hardcoded; kernel.py must be self-contained).
B = 32
S = 2048
E = 1024  # encoder hidden
O = 1024  # output dim / attention proj dim
N_CORES = 8
BL = B // N_CORES  # batches per core = 4

P = 128    # partitions
F = 512    # matmul moving free dim (one fp32 PSUM bank)
KE = E // P   # 8 contraction tiles over e
MT = O // P   # 8 output-row tiles over p
NCH = S // F  # full-width s-chunks
F2 = F // 2
# Non-uniform chunking: the last 512 splits into two 256 chunks so the
# penultimate chunk's softmax+weighted-sum hides under the last chunk's
# matmuls and only a half-size post remains exposed at the end.
CHUNKS = [(0, F), (F, F), (2 * F, F), (3 * F, F2), (3 * F + F2, F2)]
V = len(CHUNKS)

_PROGRAM = None


def _build_program():
    import concourse.tile as tile
    from concourse import bacc, mybir

    f32 = mybir.dt.float32
    f32r = mybir.dt.float32r
    AF = mybir.ActivationFunctionType
    AX = mybir.AxisListType
    ALU = mybir.AluOpType

    nc = bacc.Bacc("TRN2", target_bir_lowering=False, debug=False,
                   num_devices=N_CORES)

    encT = nc.dram_tensor("encT", [BL, E, S], f32r, kind="ExternalInput").ap()
    weT = nc.dram_tensor("weT", [P, KE, O], f32r, kind="ExternalInput").ap()
    hb = nc.dram_tensor("hb", [P, MT, BL], f32, kind="ExternalInput").ap()
    vm = nc.dram_tensor("vm", [P, MT], f32r, kind="ExternalInput").ap()
    wz = nc.dram_tensor("wz", [P, F], f32r, kind="ExternalInput").ap()
    # out[b, ep, kt] = weighted[b, kt*128 + ep]; host transposes back.
    out = nc.dram_tensor("out", [BL, P, KE], f32, kind="ExternalOutput").ap()

    with tile.TileContext(nc) as tc, ExitStack() as ctx:
        consts = ctx.enter_context(tc.tile_pool(name="consts", bufs=1))
        enc_pool = ctx.enter_context(tc.tile_pool(name="enc", bufs=5))
        epool = ctx.enter_context(tc.tile_pool(name="energy", bufs=6))
        spool = ctx.enter_context(tc.tile_pool(name="scores", bufs=3))
        bpool = ctx.enter_context(tc.tile_pool(name="bcast", bufs=3))
        jpool = ctx.enter_context(tc.tile_pool(name="junk", bufs=2))
        acpool = ctx.enter_context(tc.tile_pool(name="acc", bufs=2))
        opool = ctx.enter_context(tc.tile_pool(name="outsb", bufs=2))
        small = ctx.enter_context(tc.tile_pool(name="small", bufs=8))
        pps = ctx.enter_context(tc.tile_pool(name="pps", bufs=8, space="PSUM"))

        def ps_tile():
            return pps.tile([P, F], f32, tag="ps", name="ps")

        weT_sb = consts.tile([P, KE, O], f32r)
        vm_sb = consts.tile([P, MT], f32r)
        hb_sb = consts.tile([P, MT, BL], f32)

        def load_chunk(b, lo, ln):
            # one contiguous [P, KE, F] tile per s-chunk: slice k feeds the
            # pass-A matmuls; the whole tile feeds the DVE weighted sum.
            t = enc_pool.tile([P, KE, F], f32r, tag="ech")
            for k in range(KE):
                nc.sync.dma_start(
                    t[:, k, :ln], encT[b, k * P:(k + 1) * P, lo:lo + ln])
            return t

        class BState:
            pass

        def b_begin(b):
            st = BState()
            st.nmrow = small.tile([1, V], f32, tag="nmrow", name="nmrow")
            st.denrow = small.tile([1, V], f32, tag="denrow", name="denrow")
            st.acc = acpool.tile([P, KE, V], f32, tag="acc", name="acc")
            return st

        def post_part(st, echunk, score_ap, vc, ln, split_reduce=False):
            # chunk(-half)-local softmax + weighted partial sum on
            # ACT/DVE/GpSimd, reading the score psum bank directly.
            nc.vector.reduce_max(st.nmrow[:, vc:vc + 1], score_ap,
                                 axis=AX.X, negate=True)
            erow = spool.tile([1, F], f32, tag="erow", name="erow")
            nc.scalar.activation(erow[:, :ln], score_ap, AF.Exp,
                                 bias=st.nmrow[:, vc:vc + 1],
                                 accum_out=st.denrow[:, vc:vc + 1])
            erow_bc = bpool.tile([P, F], f32, tag="erow_bc", name="erow_bc")
            nc.gpsimd.partition_broadcast(erow_bc[:, :ln], erow[:, :ln])
            prod = jpool.tile([P, KE, F], f32, tag="junk", name="prod")
            nc.vector.tensor_tensor(
                prod[:, :, :ln], echunk[:, :, :ln].bitcast(f32),
                erow_bc[:, None, :ln].to_broadcast((P, KE, ln)), ALU.mult)
            if not split_reduce:
                nc.vector.reduce_sum(st.acc[:, :, vc], prod[:, :, :ln],
                                     axis=AX.X)
            else:
                # exposed tail: DVE reduces k0-3 while ACT accumulates k4-7
                KH = KE // 2
                nc.vector.reduce_sum(st.acc[:, :KH, vc],
                                     prod[:, :KH, :ln], axis=AX.X)
                for k in range(KH, KE):
                    j2 = jpool.tile([P, F], f32, tag="junk2", name="j2")
                    nc.scalar.activation(
                        j2[:, :ln], prod[:, k, :ln], AF.Copy,
                        accum_out=st.acc[:, k, vc:vc + 1])

        def chunk_compute(b, vc, st, echunk, ln):
            # --- PE: energy + scores for this chunk ---
            energies = []
            for m in range(MT):
                ps = ps_tile()
                for k in range(KE):
                    nc.tensor.matmul(
                        ps[:, :ln], weT_sb[:, k, m * P:(m + 1) * P],
                        echunk[:, k, :ln], start=(k == 0), stop=(k == KE - 1))
                energy = epool.tile([P, F], f32r, tag="energy")
                nc.scalar.activation(energy[:, :ln], ps[:, :ln], AF.Tanh,
                                     bias=hb_sb[:, m, b:b + 1])
                energies.append(energy)
            sps = ps_tile()
            for m in range(MT):
                nc.tensor.matmul(
                    sps[:1, :ln], vm_sb[:, m:m + 1], energies[m][:, :ln],
                    start=(m == 0), stop=(m == MT - 1))
            post_part(st, echunk, sps[:1, :ln], vc, ln,
                      split_reduce=(vc == V - 1))

        def b_end(b, st):
            # recombine the chunk-local partials: out = sum_c acc_c *
            # exp(m_c - M) / den_total.
            nmM = small.tile([1, 1], f32, tag="nmM", name="nmM")
            nc.vector.tensor_reduce(nmM[:], st.nmrow[:], axis=AX.X,
                                    op=ALU.min)
            wrow = small.tile([1, V], f32, tag="wrow", name="wrow")
            nc.scalar.activation(wrow[:], st.nmrow[:], AF.Exp,
                                 bias=nmM[:], scale=-1.0)
            dtmp = small.tile([1, V], f32, tag="dtmp", name="dtmp")
            nc.vector.tensor_tensor(dtmp[:], st.denrow[:], wrow[:], ALU.mult)
            den = small.tile([1, 1], f32, tag="den", name="den")
            nc.vector.reduce_sum(den[:], dtmp[:], axis=AX.X)
            rden = small.tile([1, 1], f32, tag="rden", name="rden")
            nc.vector.reciprocal(rden[:], den[:])
            wrow_bc = bpool.tile([P, V], f32, tag="wrow_bc", name="wrow_bc")
            nc.gpsimd.partition_broadcast(wrow_bc[:], wrow[:])
            rden_bc = bpool.tile([P, 1], f32, tag="rden_bc", name="rden_bc")
            nc.gpsimd.partition_broadcast(rden_bc[:], rden[:])
            nc.vector.tensor_tensor(
                st.acc[:], st.acc[:],
                wrow_bc[:, None, :].to_broadcast((P, KE, V)), ALU.mult)
            accf = acpool.tile([P, KE], f32, tag="accf", name="accf")
            nc.vector.reduce_sum(accf[:], st.acc[:], axis=AX.X)
            osb = opool.tile([P, KE], f32, tag="osb", name="osb")
            nc.scalar.activation(osb[:], accf[:], AF.Copy, scale=rden_bc[:])
            nc.sync.dma_start(out[b], osb[:])

        # PE warm-up: ~30 matmuls on zeroed tiles run while the first
        # DMAs are in flight, so the HAM clock gate reaches 2.4 GHz before
        # the first real matmul.
        wz_sb = consts.tile([P, F], f32r)
        nc.gpsimd.dma_start(wz_sb[:], wz[:])
        wps = ps_tile()
        for _ in range(9):
            nc.tensor.matmul(wps[:], wz_sb[:, :P], wz_sb[:],
                             start=True, stop=True)

        # Startup: weights ride the GpSimd DMA queue so the Sync queue
        # delivers the first encoder chunk immediately.
        ech0 = enc_pool.tile([P, KE, F], f32r, tag="ech")
        for k in range(KE):
            nc.sync.dma_start(
                ech0[:, k, :], encT[0, k * P:(k + 1) * P, 0:F])
            if k == 0:
                for m in range(MT):
                    nc.gpsimd.dma_start(weT_sb[:, 0, m * P:(m + 1) * P],
                                        weT[:, 0, m * P:(m + 1) * P])
            else:
                nc.gpsimd.dma_start(weT_sb[:, k, :], weT[:, k, :])
        nc.gpsimd.dma_start(vm_sb[:], vm[:])
        nc.gpsimd.dma_start(hb_sb[:], hb[:])

        # First chunk k-blocked over 8 psum banks: the first matmuls only
        # need weT[k0]+ech0[k0] instead of the full weight prefetch.
        st0 = b_begin(0)
        pstiles = [ps_tile() for _ in range(MT)]
        for k in range(KE):
            for m in range(MT):
                nc.tensor.matmul(
                    pstiles[m][:], weT_sb[:, k, m * P:(m + 1) * P],
                    ech0[:, k, :], start=(k == 0), stop=(k == KE - 1))
        energies = []
        for m in range(MT):
            energy = epool.tile([P, F], f32r, tag="energy")
            nc.scalar.activation(energy[:], pstiles[m][:], AF.Tanh,
                                 bias=hb_sb[:, m, 0:1])
            energies.append(energy)
        sps = ps_tile()
        for m in range(MT):
            nc.tensor.matmul(sps[:1, :], vm_sb[:, m:m + 1], energies[m][:],
                             start=(m == 0), stop=(m == MT - 1))
        post_part(st0, ech0, sps[:1, :], 0, F)

        states = {0: st0}
        for vc in range(1, V):
            lo, ln = CHUNKS[vc]
            chunk_compute(0, vc, st0, load_chunk(0, lo, ln), ln)
        for b in range(1, BL):
            states[b] = b_begin(b)
            for vc in range(V):
                lo, ln = CHUNKS[vc]
                chunk_compute(b, vc, states[b], load_chunk(b, lo, ln), ln)
            b_end(b - 1, states.pop(b - 1))
        b_end(BL - 1, states.pop(BL - 1))

    nc.compile()
    return nc


def _get_program():
    global _PROGRAM
    if _PROGRAM is None:
        _PROGRAM = _build_program()
    return _PROGRAM


def _make_in_maps(hidden, encoder_outputs, attn_w, attn_b, v):
    hidden = np.asarray(hidden, dtype=np.float32)
    enc = np.asarray(encoder_outputs, dtype=np.float32)
    attn_w = np.asarray(attn_w, dtype=np.float32)
    attn_b = np.asarray(attn_b, dtype=np.float32)
    v = np.asarray(v, dtype=np.float32)

    hb_full = hidden @ attn_w[:, :O].T + attn_b          # [B, O]
    weT = np.ascontiguousarray(
        attn_w[:, O:].T.reshape(KE, P, O).transpose(1, 0, 2))  # [P, KE, O]
    vm = np.ascontiguousarray(v.reshape(MT, P).T)        # [P, MT]

    in_maps = []
    for core in range(N_CORES):
        sl = slice(core * BL, (core + 1) * BL)
        encT_c = np.ascontiguousarray(
            enc[:, sl, :].transpose(1, 2, 0))            # [BL, E, S]
        hb_c = np.ascontiguousarray(
            hb_full[sl].T.reshape(MT, P, BL).transpose(1, 0, 2))  # [P, MT, BL]
        in_maps.append({
            "encT": encT_c,
            "weT": weT,
            "hb": hb_c,
            "vm": vm,
            "wz": np.zeros((P, F), dtype=np.float32),
        })
    return in_maps


def run(trace=False, **inputs):
    from concourse.bass_utils import run_bass_kernel_spmd
    nc = _get_program()
    in_maps = _make_in_maps(**inputs)
    res = run_bass_kernel_spmd(nc, in_maps, list(range(N_CORES)), trace=trace)
    # out[b, ep, kt] -> weighted[b, kt*128 + ep]
    outp = np.concatenate(
        [res.results[i]["out"].transpose(0, 2, 1).reshape(BL, O)
         for i in range(N_CORES)], axis=0)
    return outp, res


def kernel(**inputs) -> np.ndarray:
    outp, _ = run(trace=False, **inputs)
    return outp
